# revision 1
# baseline (speedup 1.0000x reference)
"""Trainium2 Bass kernel for nn_ArtifactModel_14620068675855 (moe_routing).

Model: B=262144 rows through agg MLP 256->256->256->256->1 (relu), then a
per-variant-type calibration MLP (3->12->12->1, T=5 types x 2 monotonicity
branches, monotone clip activation), branch selected by sign(logit), type
selected by one-hot(variant_types).

Strategy: pure data parallel over 8 NeuronCores (batch sharded 8 x 32768),
two NEFFs:

NEFF1 (float32r = tf32-rate matmuls, 1 cyc/column): per core, B processed
in 512-column chunks in a feature-on-partition layout ([256, B] activations,
transposed on host):
  - agg layers as 128x128-tile matmuls,
  - relu+bias fused into the PSUM->SBUF evacuation (split ACT/VectorE),
  - calibration layer 1 fused with agg layer 4: one stationary matrix maps
    h3 (256) -> 120 cal pre-activations + a logit channel + a const-1
    channel; tanh count features enter via a second accumulating matmul,
  - monotone activation = per-partition clip (one tensor_scalar max+min),
    logit/const channels ride through via (-inf,inf)/(1,1) bounds,
  - cal layers 2/3 as block-diagonal matmuls carrying logit/const along,
  - tail: one_hot-masked multiply (logit rides along via a ones row in the
    mask), then an [11->3] matmul emits out3 = [branch0, branch1, logit].

tf32 logits can flip the branch for rows with |logit| ~< 1.3e-3; a flip is
an O(1) output error. So the host takes rows with |logit_tf32| < TAU and
NEFF2 (true fp32 matmuls) recomputes exact logits for just those rows
(~1% of B); the final branch select is where(logit > 0, out0, out1).
"""

import os
import sys

sys.path.insert(0, "/opt/trn_rl_repo")
os.environ.setdefault("MYCRO_LOCAL_CACHE", "1")

import numpy as np

B = 262144
F = 256
NCORES = 8
BS = B // NCORES  # 32768 rows per core
T = 5
RR = 120  # (t, e, o) rows: 5 * 2 * 12
RZ = 122  # + logit channel (120) + const-1 channel (121)
CH = 512  # matmul free-dim chunk (one PSUM bank of fp32)
GROUP = 2048  # DMA granularity (4 chunks)
BIG = 1.0e30
TAU = 4.0e-3  # |logit_tf32| below this -> exact fp32 recompute
CAP = 1024  # NEFF2 rows per core per round (8192 global)

_CACHE = {}


def _tf32_round(x):
    """Round fp32 to the tf32 (10-bit mantissa) grid, RNE."""
    xi = np.ascontiguousarray(x, np.float32).view(np.uint32)
    r = (xi + np.uint32(0xFFF) + ((xi >> np.uint32(13)) & np.uint32(1))) & np.uint32(
        0xFFFFE000
    )
    return r.view(np.float32)


def build_neff1(bs=BS):
    """tf32 full pipeline -> out3 [3, bs]: rows 0/1 = branch values, 2 = logit.

    512-column chunks (one PSUM bank each); per chunk 18 matmuls + 6
    elementwise PSUM evacuations split across ScalarE/VectorE. Constant
    DMAs go on the scalar HWDGE ring so the sync ring starts streaming
    rep tiles immediately.
    """
    from contextlib import ExitStack

    from concourse import bacc, mybir, tile

    dt = mybir.dt
    f32 = dt.float32
    f32r = dt.float32r
    AF = mybir.ActivationFunctionType
    OP = mybir.AluOpType

    PW = 2 * CH  # 1024-column pair
    ngroup = bs // GROUP
    ppg = GROUP // PW  # pairs per group

    nc = bacc.Bacc("TRN2", target_bir_lowering=False, debug=False, num_devices=NCORES)

    def din(name, shape, d=f32):
        return nc.dram_tensor(name, shape, d, kind="ExternalInput").ap()

    rep_t = din("rep_t", [F, bs], f32r)
    counts = din("counts11", [11, bs])
    oh = din("oh11", [11, bs], f32r)
    w0t = din("w0t", [F, F], f32r)
    w1t = din("w1t", [F, F], f32r)
    w2t = din("w2t", [F, F], f32r)
    a2w = din("a2w", [F, RZ], f32r)
    reffw = din("reffw", [11, RZ], f32r)
    c2w = din("c2w", [RZ, RZ], f32r)
    c3w = din("c3w", [RZ, 11], f32r)
    selw = din("selw", [11, 3], f32r)
    low = din("low", [RZ, 1])
    highw = din("highw", [RZ, 1])
    s11 = din("s11", [11, 1])
    biasw = din("biasw", [128, 6])
    out3 = nc.dram_tensor("out3", [3, bs], f32, kind="ExternalOutput").ap()

    with tile.TileContext(nc) as tc, ExitStack() as ctx:
        cp = ctx.enter_context(tc.tile_pool(name="const", bufs=1))
        wk = {}
        for nm, src in (("w0", w0t), ("w1", w1t), ("w2", w2t)):
            for k in range(2):
                t_ = cp.tile([128, F], f32r, tag=f"{nm}k{k}")
                nc.scalar.dma_start(out=t_, in_=src[k * 128 : (k + 1) * 128, :])
                wk[(nm, k)] = t_
        a2k = []
        for k in range(2):
            t_ = cp.tile([128, RZ], f32r, tag=f"a2k{k}")
            nc.scalar.dma_start(out=t_, in_=a2w[k * 128 : (k + 1) * 128, :])
            a2k.append(t_)
        reff_t = cp.tile([11, RZ], f32r, tag="refft")
        nc.scalar.dma_start(out=reff_t, in_=reffw)
        c2_t = cp.tile([RZ, RZ], f32r, tag="c2t")
        nc.scalar.dma_start(out=c2_t, in_=c2w)
        c3_t = cp.tile([RZ, 11], f32r, tag="c3t")
        nc.scalar.dma_start(out=c3_t, in_=c3w)
        sel_t = cp.tile([11, 3], f32r, tag="selt")
        nc.scalar.dma_start(out=sel_t, in_=selw)
        lo_t = cp.tile([RZ, 1], f32, tag="lot")
        nc.scalar.dma_start(out=lo_t, in_=low)
        hi_t = cp.tile([RZ, 1], f32, tag="hit")
        nc.scalar.dma_start(out=hi_t, in_=highw)
        s11_t = cp.tile([11, 1], f32, tag="s11t")
        nc.scalar.dma_start(out=s11_t, in_=s11)
        bias_t = cp.tile([128, 6], f32, tag="biast")
        nc.scalar.dma_start(out=bias_t, in_=biasw)

        rep_p = ctx.enter_context(tc.tile_pool(name="rep", bufs=3))
        io_p = ctx.enter_context(tc.tile_pool(name="io", bufs=3))
        h_p = ctx.enter_context(tc.tile_pool(name="h", bufs=3))
        a_p = ctx.enter_context(tc.tile_pool(name="a", bufs=3))
        s_p = ctx.enter_context(tc.tile_pool(name="s", bufs=3))
        ph_p = ctx.enter_context(tc.tile_pool(name="ph", bufs=4, space="PSUM"))
        pz_p = ctx.enter_context(tc.tile_pool(name="pz", bufs=2, space="PSUM"))
        pt_p = ctx.enter_context(tc.tile_pool(name="pt", bufs=1, space="PSUM"))

        for g in range(ngroup):
            g0 = g * GROUP
            rep0 = rep_p.tile([128, GROUP], f32r, tag="rep0")
            nc.sync.dma_start(out=rep0, in_=rep_t[0:128, g0 : g0 + GROUP])
            rep1 = rep_p.tile([128, GROUP], f32r, tag="rep1")
            nc.sync.dma_start(out=rep1, in_=rep_t[128:256, g0 : g0 + GROUP])
            cnt = io_p.tile([11, GROUP], f32, tag="cnt")
            nc.sync.dma_start(out=cnt, in_=counts[:, g0 : g0 + GROUP])
            ohg = io_p.tile([11, GROUP], f32r, tag="ohg")
            nc.sync.dma_start(out=ohg, in_=oh[:, g0 : g0 + GROUP])
            eff = io_p.tile([11, GROUP], f32r, tag="eff")
            # rows 0-4: tanh(ref/max_ref[t]); 5-9: tanh(alt/max_alt[t]);
            # row 10: tanh(1e9) == 1.0 (constant-one feature for biases)
            nc.scalar.activation(eff, cnt, AF.Tanh, scale=s11_t[:, 0:1])

            for j in range(GROUP // CH):
                sl = slice(j * CH, (j + 1) * CH)
                reps = (rep0, rep1)
                hs = []
                # ---- agg layers 1-3 ----
                for li, wname in enumerate(("w0", "w1", "w2")):
                    src = reps if li == 0 else hs[-1]
                    pa = ph_p.tile([128, CH], f32, tag="ph")
                    pb = ph_p.tile([128, CH], f32, tag="ph")
                    for mt, pm in ((0, pa), (1, pb)):
                        for k in range(2):
                            rhs = src[k][:, sl] if li == 0 else src[k][:, :]
                            nc.tensor.matmul(
                                out=pm,
                                lhsT=wk[(wname, k)][:, mt * 128 : (mt + 1) * 128],
                                rhs=rhs,
                                start=(k == 0),
                                stop=(k == 1),
                            )
                    ha = h_p.tile([128, CH], f32r, tag=f"h{li}a")
                    hb = h_p.tile([128, CH], f32r, tag=f"h{li}b")
                    # relu + bias fused into the PSUM evacuation; split so
                    # DVE (which also owns clips + mask) gets 2 of 6 passes
                    nc.scalar.activation(
                        ha, pa, AF.Relu, bias=bias_t[:, 2 * li : 2 * li + 1]
                    )
                    nc.vector.tensor_scalar(
                        hb,
                        pb,
                        bias_t[:, 2 * li + 1 : 2 * li + 2],
                        0.0,
                        OP.add,
                        OP.max,
                    )
                    hs.append((ha, hb))

                h3a, h3b = hs[2]
                # ---- agg layer 4 + cal layer 1 (fused) ----
                pz1 = pz_p.tile([RZ, CH], f32, tag="pz")
                nc.tensor.matmul(
                    out=pz1, lhsT=a2k[0], rhs=h3a[:, :], start=True, stop=False
                )
                nc.tensor.matmul(
                    out=pz1, lhsT=a2k[1], rhs=h3b[:, :], start=False, stop=False
                )
                nc.tensor.matmul(
                    out=pz1, lhsT=reff_t, rhs=eff[:, sl], start=False, stop=True
                )
                # monotone activation: per-partition clip; row 120 (logit)
                # passes through, row 121 clamps to exactly 1.0
                a1 = a_p.tile([RZ, CH], f32r, tag="a1")
                nc.vector.tensor_scalar(
                    a1, pz1, lo_t[:, 0:1], hi_t[:, 0:1], OP.max, OP.min
                )
                # ---- cal layer 2 ----
                pz2 = pz_p.tile([RZ, CH], f32, tag="pz")
                nc.tensor.matmul(out=pz2, lhsT=c2_t, rhs=a1, start=True, stop=True)
                a2 = a_p.tile([RZ, CH], f32r, tag="a2")
                nc.vector.tensor_scalar(
                    a2, pz2, lo_t[:, 0:1], hi_t[:, 0:1], OP.max, OP.min
                )
                # ---- cal layer 3: rows 0-9 = z3[te], row 10 = logit ----
                pz3 = pt_p.tile([11, CH], f32, tag="pz3")
                nc.tensor.matmul(out=pz3, lhsT=c3_t, rhs=a2, start=True, stop=True)
                # one_hot mask rows 0-9; logit row passes via oh row 10=1
                zm = s_p.tile([11, CH], f32r, tag="zm")
                nc.vector.tensor_tensor(out=zm, in0=pz3, in1=ohg[:, sl], op=OP.mult)
                # rows 0/1/2 = branch-0 value, branch-1 value, logit
                po = pt_p.tile([3, CH], f32, tag="po")
                nc.tensor.matmul(out=po, lhsT=sel_t, rhs=zm, start=True, stop=True)
                osb = s_p.tile([3, CH], f32, tag="osb")
                nc.scalar.copy(out=osb, in_=po)
                nc.sync.dma_start(
                    out=out3[:, g0 + j * CH : g0 + (j + 1) * CH], in_=osb
                )

    nc.compile()
    return nc


def build_neff2(cap=CAP):
    """Exact fp32 agg MLP -> logits for the gathered ambiguous rows."""
    from contextlib import ExitStack

    from concourse import bacc, mybir, tile

    dt = mybir.dt
    f32 = dt.float32
    AF = mybir.ActivationFunctionType
    OP = mybir.AluOpType

    nc = bacc.Bacc("TRN2", target_bir_lowering=False, debug=False, num_devices=NCORES)
    rep_t = nc.dram_tensor("rep_g", [F, cap], f32, kind="ExternalInput").ap()
    w0t = nc.dram_tensor("w0t", [F, F], f32, kind="ExternalInput").ap()
    w1t = nc.dram_tensor("w1t", [F, F], f32, kind="ExternalInput").ap()
    w2t = nc.dram_tensor("w2t", [F, F], f32, kind="ExternalInput").ap()
    w3 = nc.dram_tensor("w3", [F, 1], f32, kind="ExternalInput").ap()
    biasw = nc.dram_tensor("biasw", [128, 7], f32, kind="ExternalInput").ap()
    lout = nc.dram_tensor("logit", [1, cap], f32, kind="ExternalOutput").ap()

    with tile.TileContext(nc) as tc, ExitStack() as ctx:
        cp = ctx.enter_context(tc.tile_pool(name="const", bufs=1))
        wk = {}
        for nm, src in (("w0", w0t), ("w1", w1t), ("w2", w2t)):
            for k in range(2):
                t_ = cp.tile([128, F], f32, tag=f"{nm}k{k}")
                nc.sync.dma_start(out=t_, in_=src[k * 128 : (k + 1) * 128, :])
                wk[(nm, k)] = t_
        w3k = []
        for k in range(2):
            t_ = cp.tile([128, 1], f32, tag=f"w3k{k}")
            nc.sync.dma_start(out=t_, in_=w3[k * 128 : (k + 1) * 128, :])
            w3k.append(t_)
        bias_t = cp.tile([128, 7], f32, tag="biast")
        nc.sync.dma_start(out=bias_t, in_=biasw)

        rep_p = ctx.enter_context(tc.tile_pool(name="rep", bufs=3))
        h_p = ctx.enter_context(tc.tile_pool(name="h", bufs=3))
        s_p = ctx.enter_context(tc.tile_pool(name="s", bufs=3))
        ph_p = ctx.enter_context(tc.tile_pool(name="ph", bufs=4, space="PSUM"))
        pl_p = ctx.enter_context(tc.tile_pool(name="pl", bufs=2, space="PSUM"))

        for j in range(cap // CH):
            sl = slice(j * CH, (j + 1) * CH)
            rep0 = rep_p.tile([128, CH], f32, tag="rep0")
            nc.sync.dma_start(out=rep0, in_=rep_t[0:128, sl])
            rep1 = rep_p.tile([128, CH], f32, tag="rep1")
            nc.sync.dma_start(out=rep1, in_=rep_t[128:256, sl])
            hs = []
            for li, wname in enumerate(("w0", "w1", "w2")):
                src = (rep0, rep1) if li == 0 else hs[-1]
                pa = ph_p.tile([128, CH], f32, tag="ph")
                pb = ph_p.tile([128, CH], f32, tag="ph")
                for mt, pm in ((0, pa), (1, pb)):
                    for k in range(2):
                        nc.tensor.matmul(
                            out=pm,
                            lhsT=wk[(wname, k)][:, mt * 128 : (mt + 1) * 128],
                            rhs=src[k][:, :],
                            start=(k == 0),
                            stop=(k == 1),
                        )
                ha = h_p.tile([128, CH], f32, tag=f"h{li}a")
                hb = h_p.tile([128, CH], f32, tag=f"h{li}b")
                nc.scalar.activation(
                    ha, pa, AF.Relu, bias=bias_t[:, 2 * li : 2 * li + 1]
                )
                nc.vector.tensor_scalar(
                    hb, pb, bias_t[:, 2 * li + 1 : 2 * li + 2], 0.0, OP.add, OP.max
                )
                hs.append((ha, hb))
            h3a, h3b = hs[2]
            pl = pl_p.tile([1, CH], f32, tag="pl")
            nc.tensor.matmul(out=pl, lhsT=w3k[0], rhs=h3a, start=True, stop=False)
            nc.tensor.matmul(out=pl, lhsT=w3k[1], rhs=h3b, start=False, stop=True)
            losb = s_p.tile([1, CH], f32, tag="losb")
            # + agg_b3 via the activation bias (biasw col 6 row 0)
            nc.scalar.activation(
                losb, pl, AF.Identity, bias=bias_t[0:1, 6:7]
            )
            nc.sync.dma_start(out=lout[0:1, sl], in_=losb)

    nc.compile()
    return nc


def _prep_shared(inputs):
    """Host-side constant matrices (tiny, O(model params))."""
    f = np.float32
    g = lambda k: np.asarray(inputs[k], f)
    agg_W3, agg_b3 = g("agg_W3"), g("agg_b3")
    cal_W0, cal_b0 = g("cal_W0"), g("cal_b0")
    cal_W1, cal_b1 = g("cal_W1"), g("cal_b1")
    cal_W2, cal_b2 = g("cal_W2"), g("cal_b2")
    max_ref, max_alt = g("max_ref"), g("max_alt")

    a0 = np.abs(cal_W0)  # [T,2,12,3]
    sgn_e = np.array([1.0, -1.0], f)

    A2 = np.zeros((F, RZ), f)
    A2[:, :RR] = agg_W3[0][:, None] * a0[..., 0].reshape(-1)[None, :]
    A2[:, RR] = agg_W3[0]

    Reff = np.zeros((11, RZ), f)
    C2 = np.zeros((RZ, RZ), f)
    C3 = np.zeros((RZ, 11), f)
    for t in range(T):
        for e in range(2):
            te = t * 2 + e
            rs = slice(te * 12, te * 12 + 12)
            Reff[t, rs] = a0[t, e, :, 1] * sgn_e[e]
            Reff[5 + t, rs] = a0[t, e, :, 2] * sgn_e[e]
            Reff[10, rs] = cal_b0[t, e, :] + a0[t, e, :, 0] * agg_b3[0]
            C2[rs, rs] = np.abs(cal_W1[t, e]).T  # [o_in, o_out]
            C2[121, rs] = cal_b1[t, e, :]
            C3[rs, te] = np.abs(cal_W2[t, e, 0, :])
            C3[121, te] = cal_b2[t, e, 0]
    Reff[10, RR] = agg_b3[0]
    Reff[10, 121] = 1.0
    C2[120, 120] = 1.0
    C2[121, 121] = 1.0
    C3[120, 10] = 1.0

    lo = np.zeros((RZ, 1), f)
    hi = np.zeros((RZ, 1), f)
    opat = np.arange(12)
    lo_pat = np.where(opat < 4, 0.0, np.where(opat < 8, -BIG, -1.0))
    hi_pat = np.where(opat < 4, BIG, np.where(opat < 8, 0.0, 1.0))
    lo[:RR, 0] = np.tile(lo_pat, 10)
    hi[:RR, 0] = np.tile(hi_pat, 10)
    lo[120, 0], hi[120, 0] = -BIG, BIG
    lo[121, 0], hi[121, 0] = 1.0, 1.0

    selw = np.zeros((11, 3), f)
    selw[0:10:2, 0] = 1.0
    selw[1:10:2, 1] = 1.0
    selw[10, 2] = 1.0

    shared = {
        "w0t": _tf32_round(np.ascontiguousarray(g("agg_W0").T)),
        "w1t": _tf32_round(np.ascontiguousarray(g("agg_W1").T)),
        "w2t": _tf32_round(np.ascontiguousarray(g("agg_W2").T)),
        "a2w": _tf32_round(A2),
        "reffw": _tf32_round(Reff),
        "c2w": _tf32_round(C2),
        "c3w": _tf32_round(C3),
        "selw": selw,
        "low": lo,
        "highw": hi,
        "s11": np.concatenate([1.0 / max_ref, 1.0 / max_alt, [1.0]]).astype(f)[
            :, None
        ],
    }
    biasw = np.zeros((128, 6), f)
    for li, key in enumerate(("agg_b0", "agg_b1", "agg_b2")):
        bb = g(key)
        biasw[:, 2 * li] = bb[0:128]
        biasw[:, 2 * li + 1] = bb[128:256]
    shared["biasw"] = biasw
    return shared


def prep_in_maps(inputs, bs=BS, ncores=NCORES):
    f = np.float32
    rep = np.asarray(inputs["representations"], f)
    ref_c = np.asarray(inputs["ref_counts"], f)
    alt_c = np.asarray(inputs["alt_counts"], f)
    vt = np.asarray(inputs["variant_types"])
    shared = _prep_shared(inputs)
    t_of_row = np.repeat(np.arange(T), 2)  # [0,0,1,1,2,2,3,3,4,4]

    in_maps = []
    for c in range(ncores):
        s = slice(c * bs, (c + 1) * bs)
        counts11 = np.empty((11, bs), f)
        counts11[0:5] = ref_c[s][None, :]
        counts11[5:10] = alt_c[s][None, :]
        counts11[10] = 1.0e9
        oh11 = np.ones((11, bs), f)
        oh11[0:10] = vt[s][None, :] == t_of_row[:, None]
        m = {
            "rep_t": _tf32_round(np.ascontiguousarray(rep[s].T)),
            "counts11": counts11,
            "oh11": oh11,
        }
        m.update(shared)
        in_maps.append(m)
    return in_maps


def prep_neff2_maps(inputs, rep_rows, cap=CAP, ncores=NCORES):
    """rep_rows: [n, F] gathered ambiguous rows (n <= cap * ncores)."""
    f = np.float32
    n = rep_rows.shape[0]
    padded = np.zeros((cap * ncores, F), f)
    padded[:n] = rep_rows
    g = lambda k: np.asarray(inputs[k], f)
    biasw = np.zeros((128, 7), f)
    for li, key in enumerate(("agg_b0", "agg_b1", "agg_b2")):
        bb = g(key)
        biasw[:, 2 * li] = bb[0:128]
        biasw[:, 2 * li + 1] = bb[128:256]
    biasw[0, 6] = g("agg_b3")[0]
    shared = {
        "w0t": np.ascontiguousarray(g("agg_W0").T),
        "w1t": np.ascontiguousarray(g("agg_W1").T),
        "w2t": np.ascontiguousarray(g("agg_W2").T),
        "w3": np.ascontiguousarray(g("agg_W3").T.reshape(F, 1)),
        "biasw": biasw,
    }
    maps = []
    for c in range(ncores):
        m = {"rep_g": np.ascontiguousarray(padded[c * cap : (c + 1) * cap].T)}
        m.update(shared)
        maps.append(m)
    return maps


def kernel(**inputs):
    from concourse.bass_utils import run_bass_kernel_spmd

    if "nc1" not in _CACHE:
        _CACHE["nc1"] = build_neff1(BS)
    nc1 = _CACHE["nc1"]
    in_maps = prep_in_maps(inputs)
    res1 = run_bass_kernel_spmd(nc1, in_maps, core_ids=list(range(NCORES)))
    out0 = np.concatenate([r["out3"][0] for r in res1.results])
    out1 = np.concatenate([r["out3"][1] for r in res1.results])
    logit = np.concatenate([r["out3"][2] for r in res1.results])

    # refine the sign of near-zero logits with the exact fp32 NEFF
    amb = np.where(np.abs(logit) < TAU)[0]
    if amb.size:
        if "nc2" not in _CACHE:
            _CACHE["nc2"] = build_neff2(CAP)
        nc2 = _CACHE["nc2"]
        rep = np.asarray(inputs["representations"], np.float32)
        for i in range(0, amb.size, CAP * NCORES):
            idx = amb[i : i + CAP * NCORES]
            maps2 = prep_neff2_maps(inputs, rep[idx])
            res2 = run_bass_kernel_spmd(nc2, maps2, core_ids=list(range(NCORES)))
            lg = np.concatenate([r["logit"].reshape(-1) for r in res2.results])
            logit[idx] = lg[: idx.size]

    return np.where(logit > 0.0, out0, out1).astype(np.float32)


if __name__ == "__main__":
    nc = build_neff1(GROUP)
    print("neff1 build ok")
    nc2 = build_neff2(CAP)
    print("neff2 build ok")



# revision 3
# speedup vs baseline: 1.0313x; 1.0313x over previous
"""Trainium2 Bass kernel for nn_ArtifactModel_14620068675855 (moe_routing).

Model: B=262144 rows through agg MLP 256->256->256->256->1 (relu), then a
per-variant-type calibration MLP (3->12->12->1, T=5 types x 2 monotonicity
branches, monotone clip activation), branch selected by sign(logit), type
selected by one_hot(variant_types).

v3 strategy (single NEFF, pure data parallel, batch 8 x 32768):

  * HOST sorts rows by variant type, so every 512-column chunk is (almost)
    single-type.  The fused agg-layer-4 + calibration-layer-1 stationaries
    are per-chunk (streamed from DRAM), so no one-hot masking is needed and
    the z-space is 25 rows (2 branches x 12 units + logit).
  * DEVICE does 15 f32r matmul passes per 512-col chunk: 12 for the three
    256x256 agg layers + 3 for the fused [256->25] + rank-2 count-feature
    update, then one clip (monotone activation, bias folded into shifted
    bounds) and DMAs the 25-row a1 activations out (25 x B = 26 MB total).
  * tanh count features are precomputed on host; calibration layers 2+3
    (24 + 2 units/row) run on host in exact fp32 (~0.4 GFLOP numpy).
  * Both 128-row halves of each agg layer accumulate into one [128,1024]
    2-bank PSUM tile, evacuated by a single [128,1024] relu (ScalarE for
    layers 0/2, VectorE for layer 1).  Valid because agg biases are zero;
    a split-evacuation fallback NEFF handles nonzero agg biases.
  * PSUM: 3x [128,1024] agg tiles (6 banks) + pz1 double-buffered (2) = 8.
  * Rows in mixed-type boundary chunks (<= ~1k) and rows whose tf32 logit
    is within TAU of zero (~3.4k) are recomputed / re-selected exactly on
    the host in fp32 numpy.
"""

import os
import sys

sys.path.insert(0, "/opt/trn_rl_repo")
os.environ.setdefault("MYCRO_LOCAL_CACHE", "1")

import numpy as np

B = 262144
F = 256
NCORES = 8
BS = B // NCORES  # 32768 rows per core
T = 5
RZ = 25  # z rows: 2 branches x 12 units + logit
SCOLS = 75  # statR cols per chunk: a2k0 25 | a2k1 25 | reff 25
CH = 512  # matmul free-dim chunk (one PSUM bank of fp32)
GROUP = 2048  # DMA granularity (4 chunks)
BIG = 1.0e30
TAU = 4.0e-3  # |logit_tf32| below this -> exact fp32 sign recompute (host)

_CACHE = {}


def _tf32_round(x):
    """Round fp32 to the tf32 (10-bit mantissa) grid, RNE."""
    xi = np.ascontiguousarray(x, np.float32).view(np.uint32)
    r = (xi + np.uint32(0xFFF) + ((xi >> np.uint32(13)) & np.uint32(1))) & np.uint32(
        0xFFFFE000
    )
    return r.view(np.float32)


def build_neff1(bs=BS, split_bias=False):
    """tf32 agg + fused cal-layer-1 -> a1 [25, bs] (bias-shifted clip repr)."""
    from contextlib import ExitStack

    from concourse import bacc, mybir, tile

    dt = mybir.dt
    f32 = dt.float32
    f32r = dt.float32r
    AF = mybir.ActivationFunctionType
    OP = mybir.AluOpType

    nchunk = bs // CH
    ngroup = bs // GROUP
    cpg = GROUP // CH  # chunks per group

    nc = bacc.Bacc("TRN2", target_bir_lowering=False, debug=False, num_devices=NCORES)

    def din(name, shape, d=f32):
        return nc.dram_tensor(name, shape, d, kind="ExternalInput").ap()

    rep_t = din("rep_t", [F, bs], f32r)
    eff2 = din("eff2", [2, bs], f32r)  # host-precomputed tanh count features
    statR = din("statR", [128, nchunk * SCOLS], f32r)
    statF = din("statF", [RZ, nchunk * 2])  # shifted clip bounds (lo|hi)
    w0t = din("w0t", [F, F], f32r)
    w1t = din("w1t", [F, F], f32r)
    w2t = din("w2t", [F, F], f32r)
    if split_bias:
        biasw = din("biasw", [128, 6])
    a1out = nc.dram_tensor("a1", [RZ, bs], f32, kind="ExternalOutput").ap()

    with tile.TileContext(nc) as tc, ExitStack() as ctx:
        cp = ctx.enter_context(tc.tile_pool(name="const", bufs=1))
        wk = {}
        for nm, src in (("w0", w0t), ("w1", w1t), ("w2", w2t)):
            for k in range(2):
                t_ = cp.tile([128, F], f32r, tag=f"{nm}k{k}")
                nc.scalar.dma_start(out=t_, in_=src[k * 128 : (k + 1) * 128, :])
                wk[(nm, k)] = t_
        if split_bias:
            bias_t = cp.tile([128, 6], f32, tag="biast")
            nc.scalar.dma_start(out=bias_t, in_=biasw)

        rep_p = ctx.enter_context(tc.tile_pool(name="rep", bufs=3))
        io_p = ctx.enter_context(tc.tile_pool(name="io", bufs=3))
        st_p = ctx.enter_context(tc.tile_pool(name="st", bufs=3))
        h_p = ctx.enter_context(tc.tile_pool(name="h", bufs=5))
        a_p = ctx.enter_context(tc.tile_pool(name="a", bufs=4))
        ph_p = ctx.enter_context(tc.tile_pool(name="ph", bufs=3, space="PSUM"))
        pz_p = ctx.enter_context(tc.tile_pool(name="pz", bufs=2, space="PSUM"))

        for g in range(ngroup):
            g0 = g * GROUP
            rep0 = rep_p.tile([128, GROUP], f32r, tag="rep0")
            nc.sync.dma_start(out=rep0, in_=rep_t[0:128, g0 : g0 + GROUP])
            rep1 = rep_p.tile([128, GROUP], f32r, tag="rep1")
            nc.sync.dma_start(out=rep1, in_=rep_t[128:256, g0 : g0 + GROUP])
            eff = io_p.tile([2, GROUP], f32r, tag="eff")
            nc.sync.dma_start(out=eff, in_=eff2[:, g0 : g0 + GROUP])
            stR = st_p.tile([128, cpg * SCOLS], f32r, tag="stR")
            nc.scalar.dma_start(
                out=stR, in_=statR[:, g * cpg * SCOLS : (g + 1) * cpg * SCOLS]
            )
            stF = st_p.tile([RZ, cpg * 2], f32, tag="stF")
            nc.scalar.dma_start(
                out=stF, in_=statF[:, g * cpg * 2 : (g + 1) * cpg * 2]
            )

            for j in range(cpg):
                sl = slice(j * CH, (j + 1) * CH)
                c0 = j * SCOLS
                reps = (rep0, rep1)
                hs = []
                # ---- agg layers 1-3: both halves in one [128,1024] psum ----
                for li, wname in enumerate(("w0", "w1", "w2")):
                    ph = ph_p.tile([128, 2 * CH], f32, tag="ph")
                    for mt in range(2):
                        for k in range(2):
                            if li == 0:
                                rhs = reps[k][:, sl]
                            else:
                                rhs = hs[-1][:, k * CH : (k + 1) * CH]
                            nc.tensor.matmul(
                                out=ph[:, mt * CH : (mt + 1) * CH],
                                lhsT=wk[(wname, k)][:, mt * 128 : (mt + 1) * 128],
                                rhs=rhs,
                                start=(k == 0),
                                stop=(k == 1),
                            )
                    h = h_p.tile([128, 2 * CH], f32r, tag=f"h{li}")
                    if split_bias:
                        # general agg-bias path: per-half evacuation
                        nc.scalar.activation(
                            h[:, 0:CH],
                            ph[:, 0:CH],
                            AF.Relu,
                            bias=bias_t[:, 2 * li : 2 * li + 1],
                        )
                        nc.vector.tensor_scalar(
                            h[:, CH : 2 * CH],
                            ph[:, CH : 2 * CH],
                            bias_t[:, 2 * li + 1 : 2 * li + 2],
                            0.0,
                            OP.add,
                            OP.max,
                        )
                    else:
                        # zero agg-bias fast path: one [128,1024] relu
                        if li == 1:
                            nc.vector.tensor_scalar(h, ph, 0.0, None, OP.max)
                        else:
                            nc.scalar.activation(h, ph, AF.Relu)
                    hs.append(h)

                h3 = hs[2]
                # ---- agg layer 4 + cal layer 1 (fused, 25-row z) ----
                pz = pz_p.tile([RZ, CH], f32, tag="pz")
                nc.tensor.matmul(
                    out=pz,
                    lhsT=stR[:, c0 : c0 + RZ],
                    rhs=h3[:, 0:CH],
                    start=True,
                    stop=False,
                )
                nc.tensor.matmul(
                    out=pz,
                    lhsT=stR[:, c0 + RZ : c0 + 2 * RZ],
                    rhs=h3[:, CH : 2 * CH],
                    start=False,
                    stop=False,
                )
                nc.tensor.matmul(
                    out=pz,
                    lhsT=stR[0:2, c0 + 2 * RZ : c0 + 3 * RZ],
                    rhs=eff[:, sl],
                    start=False,
                    stop=True,
                )
                # monotone activation: per-partition clip with bias-shifted
                # bounds; logit row 24 rides through via (-BIG, BIG)
                a1 = a_p.tile([RZ, CH], f32, tag="a1")
                nc.vector.tensor_scalar(
                    a1,
                    pz,
                    stF[:, 2 * j : 2 * j + 1],
                    stF[:, 2 * j + 1 : 2 * j + 2],
                    OP.max,
                    OP.min,
                )
                nc.sync.dma_start(
                    out=a1out[:, g0 + j * CH : g0 + (j + 1) * CH], in_=a1
                )

    nc.compile()
    return nc


def _type_templates(inputs):
    """Per-variant-type stage-1 stationaries + bias-shifted clip bounds."""
    f = np.float32
    g = lambda k: np.asarray(inputs[k], f)
    agg_W3, agg_b3 = g("agg_W3"), g("agg_b3")
    cal_W0, cal_b0 = g("cal_W0"), g("cal_b0")
    sgn_e = np.array([1.0, -1.0], f)
    opat = np.arange(12)
    lo_pat = np.where(opat < 4, 0.0, np.where(opat < 8, -BIG, -1.0)).astype(f)
    hi_pat = np.where(opat < 4, BIG, np.where(opat < 8, 0.0, 1.0)).astype(f)
    lo_z = np.concatenate([lo_pat, lo_pat, [-BIG]]).astype(f)
    hi_z = np.concatenate([hi_pat, hi_pat, [BIG]]).astype(f)

    stRs, stFs, b1s = [], [], []
    for t in range(T):
        a0 = np.abs(cal_W0[t])  # [2,12,3]
        A2 = np.zeros((F, RZ), f)
        Reff = np.zeros((2, RZ), f)
        b1 = np.zeros(RZ, f)
        for e in range(2):
            rs = slice(e * 12, e * 12 + 12)
            A2[:, rs] = agg_W3[0][:, None] * a0[e, :, 0][None, :]
            Reff[0, rs] = a0[e, :, 1] * sgn_e[e]
            Reff[1, rs] = a0[e, :, 2] * sgn_e[e]
            b1[rs] = cal_b0[t, e, :] + a0[e, :, 0] * agg_b3[0]
        A2[:, 24] = agg_W3[0]
        b1[24] = agg_b3[0]
        stR = np.zeros((128, SCOLS), f)
        stR[:, 0:RZ] = A2[0:128]
        stR[:, RZ : 2 * RZ] = A2[128:256]
        stR[0:2, 2 * RZ : 3 * RZ] = Reff
        stF = np.stack(
            [np.clip(lo_z - b1, -BIG, BIG), np.clip(hi_z - b1, -BIG, BIG)], axis=1
        ).astype(f)  # [RZ, 2]
        stRs.append(_tf32_round(stR))
        stFs.append(stF)
        b1s.append(b1)
    return stRs, stFs, np.stack(b1s)


def _host_prep(inputs):
    """Sort by type, build per-core input maps + repair metadata."""
    f = np.float32
    rep = np.asarray(inputs["representations"], f)
    refc = np.asarray(inputs["ref_counts"], f)
    altc = np.asarray(inputs["alt_counts"], f)
    vt = np.asarray(inputs["variant_types"])
    max_ref = np.asarray(inputs["max_ref"], f)
    max_alt = np.asarray(inputs["max_alt"], f)

    perm = np.argsort(vt, kind="stable")
    reps, refs, alts, vts = rep[perm], refc[perm], altc[perm], vt[perm]
    nchunk = B // CH
    tch = vts[::CH].copy()
    mixed_idx = np.where(vts != np.repeat(tch, CH))[0]

    stRs, stFs, b1s = _type_templates(inputs)
    statR = np.concatenate([stRs[t] for t in tch], axis=1)  # [128, nchunk*SCOLS]
    statF = np.concatenate([stFs[t] for t in tch], axis=1)  # [RZ, nchunk*2]

    eff2 = np.empty((2, B), f)
    eff2[0] = np.tanh(refs * np.repeat(1.0 / max_ref[tch], CH))
    eff2[1] = np.tanh(alts * np.repeat(1.0 / max_alt[tch], CH))
    eff2 = _tf32_round(eff2)

    rep_t_all = _tf32_round(np.ascontiguousarray(reps.T))

    zero_bias = not any(np.any(np.asarray(inputs[f"agg_b{i}"])) for i in range(3))

    in_maps = []
    cpc = BS // CH  # chunks per core
    for c in range(NCORES):
        s = slice(c * BS, (c + 1) * BS)
        m = {
            "rep_t": np.ascontiguousarray(rep_t_all[:, s]),
            "eff2": np.ascontiguousarray(eff2[:, s]),
            "statR": np.ascontiguousarray(
                statR[:, c * cpc * SCOLS : (c + 1) * cpc * SCOLS]
            ),
            "statF": np.ascontiguousarray(statF[:, c * cpc * 2 : (c + 1) * cpc * 2]),
            "w0t": _tf32_round(np.ascontiguousarray(np.asarray(inputs["agg_W0"], f).T)),
            "w1t": _tf32_round(np.ascontiguousarray(np.asarray(inputs["agg_W1"], f).T)),
            "w2t": _tf32_round(np.ascontiguousarray(np.asarray(inputs["agg_W2"], f).T)),
        }
        if not zero_bias:
            biasw = np.zeros((128, 6), f)
            for li in range(3):
                bb = np.asarray(inputs[f"agg_b{li}"], f)
                biasw[:, 2 * li] = bb[0:128]
                biasw[:, 2 * li + 1] = bb[128:256]
            m["biasw"] = biasw
        in_maps.append(m)

    meta = dict(
        perm=perm, reps=reps, refs=refs, alts=alts, vts=vts, tch=tch, b1s=b1s,
        mixed_idx=mixed_idx, zero_bias=zero_bias,
        max_ref=max_ref, max_alt=max_alt,
    )
    return in_maps, meta


def _host_logits(inputs, rep_rows):
    f = np.float32
    h = rep_rows
    for i in range(3):
        h = np.maximum(
            h @ np.asarray(inputs[f"agg_W{i}"], f).T
            + np.asarray(inputs[f"agg_b{i}"], f),
            0.0,
        )
    return (
        h @ np.asarray(inputs["agg_W3"], f).T[:, 0]
        + np.asarray(inputs["agg_b3"], f)[0]
    )


def _host_forward_rows(inputs, meta, rows):
    """Exact fp32 reference forward for a subset of sorted-row indices."""
    f = np.float32
    lg = _host_logits(inputs, meta["reps"][rows])
    t_m = meta["vts"][rows]
    xr = np.tanh(meta["refs"][rows] / meta["max_ref"][t_m])
    xa = np.tanh(meta["alts"][rows] / meta["max_alt"][t_m])
    x = np.stack([lg, xr, xa], -1)
    signs = np.array([[1, 1, 1], [1, -1, -1]], f)
    z = x[:, None, :] * signs[None, :, :]  # [n,2,3]
    for i in range(3):
        Wc = np.abs(np.asarray(inputs[f"cal_W{i}"], f))[t_m]  # [n,2,o,i]
        bc = np.asarray(inputs[f"cal_b{i}"], f)[t_m]
        z = np.einsum("nei,neoi->neo", z, Wc) + bc
        if i < 2:
            s = z.shape[-1] // 3
            z = np.concatenate(
                [
                    np.maximum(z[..., :s], 0),
                    np.minimum(z[..., s : 2 * s], 0),
                    np.clip(z[..., 2 * s :], -1, 1),
                ],
                -1,
            )
    return np.where(lg > 0, z[:, 0, 0], z[:, 1, 0])


def _postprocess(inputs, meta, results):
    """Host cal layers 2+3 in fp32, branch-select, repairs, unsort."""
    f = np.float32
    nchunk = B // CH
    tch = meta["tch"]
    a1 = np.concatenate([r["a1"] for r in results], axis=1)  # [RZ, B]
    a1c = np.ascontiguousarray(
        a1.reshape(RZ, nchunk, CH).transpose(1, 0, 2)
    )  # [nchunk, RZ, CH]
    a1c += meta["b1s"][tch][:, :, None]  # undo bias shift

    lg = np.ascontiguousarray(a1c[:, 24, :]).reshape(-1)

    cal_W1 = np.abs(np.asarray(inputs["cal_W1"], f))  # [T,2,12,12]
    cal_b1 = np.asarray(inputs["cal_b1"], f)
    cal_W2 = np.abs(np.asarray(inputs["cal_W2"], f))  # [T,2,1,12]
    cal_b2 = np.asarray(inputs["cal_b2"], f)

    # cal layer 2: block-diagonal [24 <- 24] per type
    C2h = np.zeros((T, 24, 24), f)
    for t in range(T):
        for e in range(2):
            rs = slice(e * 12, e * 12 + 12)
            C2h[t, rs, rs] = cal_W1[t, e]  # [o,i] applied to a1 block
    z2 = np.matmul(C2h[tch], a1c[:, 0:24, :])  # [nchunk, 24, CH]
    z2 += cal_b1[tch].reshape(nchunk, 24, 1)
    # monotone activation per 12-block: 0-3 relu, 4-7 -relu(-x), 8-11 clamp
    for e in range(2):
        o = e * 12
        np.maximum(z2[:, o : o + 4], 0.0, out=z2[:, o : o + 4])
        np.minimum(z2[:, o + 4 : o + 8], 0.0, out=z2[:, o + 4 : o + 8])
        np.clip(z2[:, o + 8 : o + 12], -1.0, 1.0, out=z2[:, o + 8 : o + 12])
    # cal layer 3: [2 <- 12] per branch
    z3 = np.einsum(
        "cev,cevb->ceb", cal_W2[tch][:, :, 0, :], z2.reshape(nchunk, 2, 12, CH)
    )
    z3 += cal_b2[tch].reshape(nchunk, 2, 1)
    v0 = np.ascontiguousarray(z3[:, 0, :]).reshape(-1)
    v1 = np.ascontiguousarray(z3[:, 1, :]).reshape(-1)

    out = np.where(lg > 0.0, v0, v1).astype(f)

    # exact sign for near-zero tf32 logits
    amb = np.where(np.abs(lg) < TAU)[0]
    if amb.size:
        lgx = _host_logits(inputs, meta["reps"][amb])
        out[amb] = np.where(lgx > 0.0, v0[amb], v1[amb])

    # exact values for rows whose chunk used the wrong type's calibration
    midx = meta["mixed_idx"]
    if midx.size:
        out[midx] = _host_forward_rows(inputs, meta, midx)

    res = np.empty(B, f)
    res[meta["perm"]] = out
    return res


def _run(inputs, trace=False, tmpdir=None):
    from concourse.bass_utils import run_bass_kernel_spmd

    in_maps, meta = _host_prep(inputs)
    key = ("nc1", meta["zero_bias"])
    if key not in _CACHE:
        _CACHE[key] = build_neff1(BS, split_bias=not meta["zero_bias"])
    nc1 = _CACHE[key]
    kwargs = {}
    if tmpdir is not None:
        kwargs["tmpdir"] = tmpdir
    res1 = run_bass_kernel_spmd(
        nc1, in_maps, core_ids=list(range(NCORES)), trace=trace, **kwargs
    )
    out = _postprocess(inputs, meta, res1.results)
    return out, res1


def kernel(**inputs):
    out, _ = _run(inputs, trace=False)
    return out


if __name__ == "__main__":
    nc = build_neff1(GROUP)
    print("neff1 build ok")


# revision 5
# speedup vs baseline: 1.7038x; 1.6520x over previous
"""Trainium2 Bass kernel for nn_ArtifactModel_14620068675855 (moe_routing).

Model: B=262144 rows through agg MLP 256->256->256->256->1 (relu), then a
per-variant-type calibration MLP (3->12->12->1, T=5 types x 2 monotonicity
branches, monotone clip activation), branch selected by sign(logit), type
selected by one_hot(variant_types).

v3 strategy (single NEFF, pure data parallel, batch 8 x 32768):

  * HOST sorts rows by variant type, so every 512-column chunk is (almost)
    single-type.  The fused agg-layer-4 + calibration-layer-1 stationaries
    are per-chunk (streamed from DRAM), so no one-hot masking is needed and
    the z-space is 25 rows (2 branches x 12 units + logit).
  * DEVICE does 15 f32r matmul passes per 512-col chunk: 12 for the three
    256x256 agg layers + 3 for the fused [256->25] + rank-2 count-feature
    update, then one clip (monotone activation, bias folded into shifted
    bounds) and DMAs the 25-row a1 activations out (25 x B = 26 MB total).
  * tanh count features are precomputed on host; calibration layers 2+3
    (24 + 2 units/row) run on host in exact fp32 (~0.4 GFLOP numpy).
  * Both 128-row halves of each agg layer accumulate into one [128,1024]
    2-bank PSUM tile, evacuated by a single [128,1024] relu (ScalarE for
    layers 0/2, VectorE for layer 1).  Valid because agg biases are zero;
    a split-evacuation fallback NEFF handles nonzero agg biases.
  * PSUM: 3x [128,1024] agg tiles (6 banks) + pz1 double-buffered (2) = 8.
  * Rows in mixed-type boundary chunks (<= ~1k) and rows whose tf32 logit
    is within TAU of zero (~3.4k) are recomputed / re-selected exactly on
    the host in fp32 numpy.
"""

import os
import sys

sys.path.insert(0, "/opt/trn_rl_repo")
os.environ.setdefault("MYCRO_LOCAL_CACHE", "1")

import numpy as np

B = 262144
F = 256
NCORES = 8
BS = B // NCORES  # 32768 rows per core
T = 5
RZ = 25  # z rows: 2 branches x 12 units + logit
SCOLS = 75  # statR cols per chunk: a2k0 25 | a2k1 25 | reff 25
CH = 512  # matmul free-dim chunk (one PSUM bank of fp32)
GROUP = 2048  # DMA granularity (4 chunks)
BIG = 1.0e30
TAU = 4.0e-3  # |logit_tf32| below this -> exact fp32 sign recompute (host)

_CACHE = {}


def _tf32_round(x):
    """Round fp32 to the tf32 (10-bit mantissa) grid, RNE."""
    xi = np.ascontiguousarray(x, np.float32).view(np.uint32)
    r = (xi + np.uint32(0xFFF) + ((xi >> np.uint32(13)) & np.uint32(1))) & np.uint32(
        0xFFFFE000
    )
    return r.view(np.float32)


def build_neff1(bs=BS, split_bias=False):
    """tf32 agg + fused cal-layer-1 -> a1 [25, bs] (bias-shifted clip repr)."""
    from contextlib import ExitStack

    from concourse import bacc, mybir, tile

    dt = mybir.dt
    f32 = dt.float32
    f32r = dt.float32r
    AF = mybir.ActivationFunctionType
    OP = mybir.AluOpType

    nchunk = bs // CH
    ngroup = bs // GROUP
    cpg = GROUP // CH  # chunks per group

    nc = bacc.Bacc("TRN2", target_bir_lowering=False, debug=False, num_devices=NCORES)

    def din(name, shape, d=f32):
        return nc.dram_tensor(name, shape, d, kind="ExternalInput").ap()

    rep_t = din("rep_t", [F, bs], f32r)
    eff2 = din("eff2", [2, bs], f32r)  # host-precomputed tanh count features
    statR = din("statR", [128, nchunk * SCOLS], f32r)
    statF = din("statF", [RZ, nchunk * 2])  # shifted clip bounds (lo|hi)
    w0t = din("w0t", [F, F], f32r)
    w1t = din("w1t", [F, F], f32r)
    w2t = din("w2t", [F, F], f32r)
    if split_bias:
        biasw = din("biasw", [128, 6])
    a1out = nc.dram_tensor("a1", [RZ, bs], f32, kind="ExternalOutput").ap()

    with tile.TileContext(nc) as tc, ExitStack() as ctx:
        cp = ctx.enter_context(tc.tile_pool(name="const", bufs=1))
        wk = {}
        for nm, src in (("w0", w0t), ("w1", w1t), ("w2", w2t)):
            for k in range(2):
                t_ = cp.tile([128, F], f32r, tag=f"{nm}k{k}")
                nc.scalar.dma_start(out=t_, in_=src[k * 128 : (k + 1) * 128, :])
                wk[(nm, k)] = t_
        if split_bias:
            bias_t = cp.tile([128, 6], f32, tag="biast")
            nc.scalar.dma_start(out=bias_t, in_=biasw)

        rep_p = ctx.enter_context(tc.tile_pool(name="rep", bufs=3))
        io_p = ctx.enter_context(tc.tile_pool(name="io", bufs=3))
        st_p = ctx.enter_context(tc.tile_pool(name="st", bufs=3))
        h_p = ctx.enter_context(tc.tile_pool(name="h", bufs=5))
        a_p = ctx.enter_context(tc.tile_pool(name="a", bufs=4))
        ph_p = ctx.enter_context(tc.tile_pool(name="ph", bufs=3, space="PSUM"))
        pz_p = ctx.enter_context(tc.tile_pool(name="pz", bufs=2, space="PSUM"))

        gt = {}  # group -> (rep0, rep1, eff, stR, stF)

        def load_group(g):
            g0 = g * GROUP
            rep0 = rep_p.tile([128, GROUP], f32r, tag="rep0")
            nc.sync.dma_start(out=rep0, in_=rep_t[0:128, g0 : g0 + GROUP])
            rep1 = rep_p.tile([128, GROUP], f32r, tag="rep1")
            nc.sync.dma_start(out=rep1, in_=rep_t[128:256, g0 : g0 + GROUP])
            eff = io_p.tile([2, GROUP], f32r, tag="eff")
            nc.sync.dma_start(out=eff, in_=eff2[:, g0 : g0 + GROUP])
            stR = st_p.tile([128, cpg * SCOLS], f32r, tag="stR")
            nc.scalar.dma_start(
                out=stR, in_=statR[:, g * cpg * SCOLS : (g + 1) * cpg * SCOLS]
            )
            stF = st_p.tile([RZ, cpg * 2], f32, tag="stF")
            nc.scalar.dma_start(
                out=stF, in_=statF[:, g * cpg * 2 : (g + 1) * cpg * 2]
            )
            gt[g] = (rep0, rep1, eff, stR, stF)

        hrefs = {}  # chunk -> [h0, h1, h2]

        def agg_layer(c, li):
            """Stage A/B/C: one 256->256 agg layer for chunk c (4 matmuls
            into a [128,1024] 2-bank psum + one merged relu evacuation)."""
            g, j = divmod(c, cpg)
            sl = slice(j * CH, (j + 1) * CH)
            rep0, rep1, _, _, _ = gt[g]
            wname = ("w0", "w1", "w2")[li]
            ph = ph_p.tile([128, 2 * CH], f32, tag="ph")
            for mt in range(2):
                for k in range(2):
                    if li == 0:
                        rhs = (rep0, rep1)[k][:, sl]
                    else:
                        rhs = hrefs[c][li - 1][:, k * CH : (k + 1) * CH]
                    nc.tensor.matmul(
                        out=ph[:, mt * CH : (mt + 1) * CH],
                        lhsT=wk[(wname, k)][:, mt * 128 : (mt + 1) * 128],
                        rhs=rhs,
                        start=(k == 0),
                        stop=(k == 1),
                    )
            h = h_p.tile([128, 2 * CH], f32r, tag=f"h{li}")
            if split_bias:
                # general agg-bias path: per-half evacuation
                nc.scalar.activation(
                    h[:, 0:CH],
                    ph[:, 0:CH],
                    AF.Relu,
                    bias=bias_t[:, 2 * li : 2 * li + 1],
                )
                nc.vector.tensor_scalar(
                    h[:, CH : 2 * CH],
                    ph[:, CH : 2 * CH],
                    bias_t[:, 2 * li + 1 : 2 * li + 2],
                    0.0,
                    OP.add,
                    OP.max,
                )
            else:
                # zero agg-bias fast path: one [128,1024] relu
                if li == 1:
                    nc.vector.tensor_scalar(h, ph, 0.0, None, OP.max)
                else:
                    nc.scalar.activation(h, ph, AF.Relu)
            hrefs.setdefault(c, []).append(h)

        def tail(c):
            """Stage D: fused agg-layer-4 + cal-layer-1 for chunk c."""
            g, j = divmod(c, cpg)
            g0 = g * GROUP
            sl = slice(j * CH, (j + 1) * CH)
            _, _, eff, stR, stF = gt[g]
            c0 = j * SCOLS
            h3 = hrefs[c][2]
            pz = pz_p.tile([RZ, CH], f32, tag="pz")
            nc.tensor.matmul(
                out=pz,
                lhsT=stR[:, c0 : c0 + RZ],
                rhs=h3[:, 0:CH],
                start=True,
                stop=False,
            )
            nc.tensor.matmul(
                out=pz,
                lhsT=stR[:, c0 + RZ : c0 + 2 * RZ],
                rhs=h3[:, CH : 2 * CH],
                start=False,
                stop=False,
            )
            nc.tensor.matmul(
                out=pz,
                lhsT=stR[0:2, c0 + 2 * RZ : c0 + 3 * RZ],
                rhs=eff[:, sl],
                start=False,
                stop=True,
            )
            # monotone activation: per-partition clip with bias-shifted
            # bounds; logit row 24 rides through via (-BIG, BIG)
            a1 = a_p.tile([RZ, CH], f32, tag="a1")
            nc.vector.tensor_scalar(
                a1,
                pz,
                stF[:, 2 * j : 2 * j + 1],
                stF[:, 2 * j + 1 : 2 * j + 2],
                OP.max,
                OP.min,
            )
            nc.sync.dma_start(
                out=a1out[:, g0 + j * CH : g0 + (j + 1) * CH], in_=a1
            )
            del hrefs[c]

        # Depth-3 software pipeline: period j runs l0(j), tail(j-2),
        # l2(j-1), l1(j) so every relu/clip latency is hidden by
        # independent matmuls while PSUM stays within 8 banks.
        load_group(0)
        if ngroup > 1:
            load_group(1)
        for j in range(nchunk + 2):
            if j < nchunk:
                # prefetch 2 groups ahead, mid-group so the recycled slot's
                # last reader (tail of 2+ periods ago) is already emitted
                if j % cpg == 2 and (j // cpg) + 2 < ngroup:
                    load_group(j // cpg + 2)
                agg_layer(j, 0)
            if j >= 2:
                tail(j - 2)
            if 1 <= j < nchunk + 1:
                agg_layer(j - 1, 2)
            if j < nchunk:
                agg_layer(j, 1)

    nc.compile()
    return nc


def _type_templates(inputs):
    """Per-variant-type stage-1 stationaries + bias-shifted clip bounds."""
    f = np.float32
    g = lambda k: np.asarray(inputs[k], f)
    agg_W3, agg_b3 = g("agg_W3"), g("agg_b3")
    cal_W0, cal_b0 = g("cal_W0"), g("cal_b0")
    sgn_e = np.array([1.0, -1.0], f)
    opat = np.arange(12)
    lo_pat = np.where(opat < 4, 0.0, np.where(opat < 8, -BIG, -1.0)).astype(f)
    hi_pat = np.where(opat < 4, BIG, np.where(opat < 8, 0.0, 1.0)).astype(f)
    lo_z = np.concatenate([lo_pat, lo_pat, [-BIG]]).astype(f)
    hi_z = np.concatenate([hi_pat, hi_pat, [BIG]]).astype(f)

    stRs, stFs, b1s = [], [], []
    for t in range(T):
        a0 = np.abs(cal_W0[t])  # [2,12,3]
        A2 = np.zeros((F, RZ), f)
        Reff = np.zeros((2, RZ), f)
        b1 = np.zeros(RZ, f)
        for e in range(2):
            rs = slice(e * 12, e * 12 + 12)
            A2[:, rs] = agg_W3[0][:, None] * a0[e, :, 0][None, :]
            Reff[0, rs] = a0[e, :, 1] * sgn_e[e]
            Reff[1, rs] = a0[e, :, 2] * sgn_e[e]
            b1[rs] = cal_b0[t, e, :] + a0[e, :, 0] * agg_b3[0]
        A2[:, 24] = agg_W3[0]
        b1[24] = agg_b3[0]
        stR = np.zeros((128, SCOLS), f)
        stR[:, 0:RZ] = A2[0:128]
        stR[:, RZ : 2 * RZ] = A2[128:256]
        stR[0:2, 2 * RZ : 3 * RZ] = Reff
        stF = np.stack(
            [np.clip(lo_z - b1, -BIG, BIG), np.clip(hi_z - b1, -BIG, BIG)], axis=1
        ).astype(f)  # [RZ, 2]
        stRs.append(_tf32_round(stR))
        stFs.append(stF)
        b1s.append(b1)
    return stRs, stFs, np.stack(b1s)


def _host_prep(inputs):
    """Sort by type, build per-core input maps + repair metadata."""
    f = np.float32
    rep = np.asarray(inputs["representations"], f)
    refc = np.asarray(inputs["ref_counts"], f)
    altc = np.asarray(inputs["alt_counts"], f)
    vt = np.asarray(inputs["variant_types"])
    max_ref = np.asarray(inputs["max_ref"], f)
    max_alt = np.asarray(inputs["max_alt"], f)

    perm = np.argsort(vt, kind="stable")
    reps, refs, alts, vts = rep[perm], refc[perm], altc[perm], vt[perm]
    nchunk = B // CH
    tch = vts[::CH].copy()
    mixed_idx = np.where(vts != np.repeat(tch, CH))[0]

    stRs, stFs, b1s = _type_templates(inputs)
    statR = np.concatenate([stRs[t] for t in tch], axis=1)  # [128, nchunk*SCOLS]
    statF = np.concatenate([stFs[t] for t in tch], axis=1)  # [RZ, nchunk*2]

    eff2 = np.empty((2, B), f)
    eff2[0] = np.tanh(refs * np.repeat(1.0 / max_ref[tch], CH))
    eff2[1] = np.tanh(alts * np.repeat(1.0 / max_alt[tch], CH))
    eff2 = _tf32_round(eff2)

    rep_t_all = _tf32_round(np.ascontiguousarray(reps.T))

    zero_bias = not any(np.any(np.asarray(inputs[f"agg_b{i}"])) for i in range(3))

    in_maps = []
    cpc = BS // CH  # chunks per core
    for c in range(NCORES):
        s = slice(c * BS, (c + 1) * BS)
        m = {
            "rep_t": np.ascontiguousarray(rep_t_all[:, s]),
            "eff2": np.ascontiguousarray(eff2[:, s]),
            "statR": np.ascontiguousarray(
                statR[:, c * cpc * SCOLS : (c + 1) * cpc * SCOLS]
            ),
            "statF": np.ascontiguousarray(statF[:, c * cpc * 2 : (c + 1) * cpc * 2]),
            "w0t": _tf32_round(np.ascontiguousarray(np.asarray(inputs["agg_W0"], f).T)),
            "w1t": _tf32_round(np.ascontiguousarray(np.asarray(inputs["agg_W1"], f).T)),
            "w2t": _tf32_round(np.ascontiguousarray(np.asarray(inputs["agg_W2"], f).T)),
        }
        if not zero_bias:
            biasw = np.zeros((128, 6), f)
            for li in range(3):
                bb = np.asarray(inputs[f"agg_b{li}"], f)
                biasw[:, 2 * li] = bb[0:128]
                biasw[:, 2 * li + 1] = bb[128:256]
            m["biasw"] = biasw
        in_maps.append(m)

    meta = dict(
        perm=perm, reps=reps, refs=refs, alts=alts, vts=vts, tch=tch, b1s=b1s,
        mixed_idx=mixed_idx, zero_bias=zero_bias,
        max_ref=max_ref, max_alt=max_alt,
    )
    return in_maps, meta


def _host_logits(inputs, rep_rows):
    f = np.float32
    h = rep_rows
    for i in range(3):
        h = np.maximum(
            h @ np.asarray(inputs[f"agg_W{i}"], f).T
            + np.asarray(inputs[f"agg_b{i}"], f),
            0.0,
        )
    return (
        h @ np.asarray(inputs["agg_W3"], f).T[:, 0]
        + np.asarray(inputs["agg_b3"], f)[0]
    )


def _host_forward_rows(inputs, meta, rows):
    """Exact fp32 reference forward for a subset of sorted-row indices."""
    f = np.float32
    lg = _host_logits(inputs, meta["reps"][rows])
    t_m = meta["vts"][rows]
    xr = np.tanh(meta["refs"][rows] / meta["max_ref"][t_m])
    xa = np.tanh(meta["alts"][rows] / meta["max_alt"][t_m])
    x = np.stack([lg, xr, xa], -1)
    signs = np.array([[1, 1, 1], [1, -1, -1]], f)
    z = x[:, None, :] * signs[None, :, :]  # [n,2,3]
    for i in range(3):
        Wc = np.abs(np.asarray(inputs[f"cal_W{i}"], f))[t_m]  # [n,2,o,i]
        bc = np.asarray(inputs[f"cal_b{i}"], f)[t_m]
        z = np.einsum("nei,neoi->neo", z, Wc) + bc
        if i < 2:
            s = z.shape[-1] // 3
            z = np.concatenate(
                [
                    np.maximum(z[..., :s], 0),
                    np.minimum(z[..., s : 2 * s], 0),
                    np.clip(z[..., 2 * s :], -1, 1),
                ],
                -1,
            )
    return np.where(lg > 0, z[:, 0, 0], z[:, 1, 0])


def _postprocess(inputs, meta, results):
    """Host cal layers 2+3 in fp32, branch-select, repairs, unsort."""
    f = np.float32
    nchunk = B // CH
    tch = meta["tch"]
    a1 = np.concatenate([r["a1"] for r in results], axis=1)  # [RZ, B]
    a1c = np.ascontiguousarray(
        a1.reshape(RZ, nchunk, CH).transpose(1, 0, 2)
    )  # [nchunk, RZ, CH]
    a1c += meta["b1s"][tch][:, :, None]  # undo bias shift

    lg = np.ascontiguousarray(a1c[:, 24, :]).reshape(-1)

    cal_W1 = np.abs(np.asarray(inputs["cal_W1"], f))  # [T,2,12,12]
    cal_b1 = np.asarray(inputs["cal_b1"], f)
    cal_W2 = np.abs(np.asarray(inputs["cal_W2"], f))  # [T,2,1,12]
    cal_b2 = np.asarray(inputs["cal_b2"], f)

    # cal layer 2: block-diagonal [24 <- 24] per type
    C2h = np.zeros((T, 24, 24), f)
    for t in range(T):
        for e in range(2):
            rs = slice(e * 12, e * 12 + 12)
            C2h[t, rs, rs] = cal_W1[t, e]  # [o,i] applied to a1 block
    z2 = np.matmul(C2h[tch], a1c[:, 0:24, :])  # [nchunk, 24, CH]
    z2 += cal_b1[tch].reshape(nchunk, 24, 1)
    # monotone activation per 12-block: 0-3 relu, 4-7 -relu(-x), 8-11 clamp
    for e in range(2):
        o = e * 12
        np.maximum(z2[:, o : o + 4], 0.0, out=z2[:, o : o + 4])
        np.minimum(z2[:, o + 4 : o + 8], 0.0, out=z2[:, o + 4 : o + 8])
        np.clip(z2[:, o + 8 : o + 12], -1.0, 1.0, out=z2[:, o + 8 : o + 12])
    # cal layer 3: [2 <- 12] per branch
    z3 = np.einsum(
        "cev,cevb->ceb", cal_W2[tch][:, :, 0, :], z2.reshape(nchunk, 2, 12, CH)
    )
    z3 += cal_b2[tch].reshape(nchunk, 2, 1)
    v0 = np.ascontiguousarray(z3[:, 0, :]).reshape(-1)
    v1 = np.ascontiguousarray(z3[:, 1, :]).reshape(-1)

    out = np.where(lg > 0.0, v0, v1).astype(f)

    # exact sign for near-zero tf32 logits
    amb = np.where(np.abs(lg) < TAU)[0]
    if amb.size:
        lgx = _host_logits(inputs, meta["reps"][amb])
        out[amb] = np.where(lgx > 0.0, v0[amb], v1[amb])

    # exact values for rows whose chunk used the wrong type's calibration
    midx = meta["mixed_idx"]
    if midx.size:
        out[midx] = _host_forward_rows(inputs, meta, midx)

    res = np.empty(B, f)
    res[meta["perm"]] = out
    return res


def _run(inputs, trace=False, tmpdir=None):
    from concourse.bass_utils import run_bass_kernel_spmd

    in_maps, meta = _host_prep(inputs)
    key = ("nc1", meta["zero_bias"])
    if key not in _CACHE:
        _CACHE[key] = build_neff1(BS, split_bias=not meta["zero_bias"])
    nc1 = _CACHE[key]
    kwargs = {}
    if tmpdir is not None:
        kwargs["tmpdir"] = tmpdir
    res1 = run_bass_kernel_spmd(
        nc1, in_maps, core_ids=list(range(NCORES)), trace=trace, **kwargs
    )
    out = _postprocess(inputs, meta, res1.results)
    return out, res1


def kernel(**inputs):
    out, _ = _run(inputs, trace=False)
    return out


if __name__ == "__main__":
    nc = build_neff1(GROUP)
    print("neff1 build ok")


# revision 10
# speedup vs baseline: 1.7332x; 1.0172x over previous
"""Trainium2 Bass kernel for nn_ArtifactModel_14620068675855 (moe_routing).

Model: B=262144 rows through agg MLP 256->256->256->256->1 (relu), then a
per-variant-type calibration MLP (3->12->12->1, T=5 types x 2 monotonicity
branches, monotone clip activation), branch selected by sign(logit), type
selected by one_hot(variant_types).

v3 strategy (single NEFF, pure data parallel, batch 8 x 32768):

  * HOST sorts rows by variant type, so every 512-column chunk is (almost)
    single-type.  The fused agg-layer-4 + calibration-layer-1 stationaries
    are per-chunk (streamed from DRAM), so no one-hot masking is needed and
    the z-space is 25 rows (2 branches x 12 units + logit).
  * DEVICE does 15 f32r matmul passes per 512-col chunk: 12 for the three
    256x256 agg layers + 3 for the fused [256->25] + rank-2 count-feature
    update, then one clip (monotone activation, bias folded into shifted
    bounds) and DMAs the 25-row a1 activations out (25 x B = 26 MB total).
  * tanh count features are precomputed on host; calibration layers 2+3
    (24 + 2 units/row) run on host in exact fp32 (~0.4 GFLOP numpy).
  * Both 128-row halves of each agg layer accumulate into one [128,1024]
    2-bank PSUM tile, evacuated by a single [128,1024] relu (ScalarE for
    layers 0/2, VectorE for layer 1).  Valid because agg biases are zero;
    a split-evacuation fallback NEFF handles nonzero agg biases.
  * PSUM: 3x [128,1024] agg tiles (6 banks) + pz1 double-buffered (2) = 8.
  * Rows in mixed-type boundary chunks (<= ~1k) and rows whose tf32 logit
    is within TAU of zero (~3.4k) are recomputed / re-selected exactly on
    the host in fp32 numpy.
"""

import os
import sys

sys.path.insert(0, "/opt/trn_rl_repo")
os.environ.setdefault("MYCRO_LOCAL_CACHE", "1")

import numpy as np

B = 262144
F = 256
NCORES = 8
BS = B // NCORES  # 32768 rows per core
T = 5
RZ = 25  # z rows: 2 branches x 12 units + logit
SCOLS = 75  # statR cols per chunk: a2k0 25 | a2k1 25 | reff 25
CH = 512  # matmul free-dim chunk (one PSUM bank of fp32)
GROUP = 2048  # DMA granularity (4 chunks)
BIG = 1.0e30
TAU = 4.0e-3  # |logit_tf32| below this -> exact fp32 sign recompute (host)

_CACHE = {}


def _tf32_round(x):
    """Round fp32 to the tf32 (10-bit mantissa) grid, RNE."""
    xi = np.ascontiguousarray(x, np.float32).view(np.uint32)
    r = (xi + np.uint32(0xFFF) + ((xi >> np.uint32(13)) & np.uint32(1))) & np.uint32(
        0xFFFFE000
    )
    return r.view(np.float32)


def build_neff1(bs=BS, split_bias=False):
    """tf32 agg + fused cal-layer-1 -> a1 [25, bs] (bias-shifted clip repr)."""
    from contextlib import ExitStack

    from concourse import bacc, mybir, tile

    dt = mybir.dt
    f32 = dt.float32
    f32r = dt.float32r
    AF = mybir.ActivationFunctionType
    OP = mybir.AluOpType

    nchunk = bs // CH
    ngroup = bs // GROUP
    cpg = GROUP // CH  # chunks per group

    nc = bacc.Bacc("TRN2", target_bir_lowering=False, debug=False, num_devices=NCORES)

    def din(name, shape, d=f32):
        return nc.dram_tensor(name, shape, d, kind="ExternalInput").ap()

    rep_t = din("rep_t", [F, bs], f32r)
    eff2 = din("eff2", [2, bs], f32r)  # host-precomputed tanh count features
    statR = din("statR", [128, nchunk * SCOLS], f32r)
    statF = din("statF", [RZ, nchunk * 2])  # shifted clip bounds (lo|hi)
    # w0t/w1t/w2t k-halves packed side by side: one DMA loads all agg weights
    wpack = din("wpack", [128, 6 * F], f32r)
    if split_bias:
        biasw = din("biasw", [128, 6])
    a1out = nc.dram_tensor("a1", [RZ, bs], f32, kind="ExternalOutput").ap()

    with tile.TileContext(nc) as tc, ExitStack() as ctx:
        cp = ctx.enter_context(tc.tile_pool(name="const", bufs=1))
        wpk = cp.tile([128, 6 * F], f32r, tag="wpack")
        nc.scalar.dma_start(out=wpk, in_=wpack)
        wk = {}
        for li, nm in enumerate(("w0", "w1", "w2")):
            for k in range(2):
                wk[(nm, k)] = wpk[:, (2 * li + k) * F : (2 * li + k + 1) * F]
        if split_bias:
            bias_t = cp.tile([128, 6], f32, tag="biast")
            nc.scalar.dma_start(out=bias_t, in_=biasw)

        rep_p = ctx.enter_context(tc.tile_pool(name="rep", bufs=3))
        io_p = ctx.enter_context(tc.tile_pool(name="io", bufs=3))
        st_p = ctx.enter_context(tc.tile_pool(name="st", bufs=3))
        h_p = ctx.enter_context(tc.tile_pool(name="h", bufs=5))
        a_p = ctx.enter_context(tc.tile_pool(name="a", bufs=4))
        ph_p = ctx.enter_context(tc.tile_pool(name="ph", bufs=3, space="PSUM"))
        pz_p = ctx.enter_context(tc.tile_pool(name="pz", bufs=2, space="PSUM"))

        gt = {}  # group -> (rep0, rep1, eff, stR, stF)

        def load_group(g):
            g0 = g * GROUP
            HG = GROUP // 2
            rep0 = rep_p.tile([128, GROUP], f32r, tag="rep0")
            rep1 = rep_p.tile([128, GROUP], f32r, tag="rep1")
            # interleaved halves so the first chunks' data lands earliest
            nc.sync.dma_start(out=rep0[:, 0:HG], in_=rep_t[0:128, g0 : g0 + HG])
            nc.sync.dma_start(out=rep1[:, 0:HG], in_=rep_t[128:256, g0 : g0 + HG])
            nc.sync.dma_start(
                out=rep0[:, HG:GROUP], in_=rep_t[0:128, g0 + HG : g0 + GROUP]
            )
            nc.sync.dma_start(
                out=rep1[:, HG:GROUP], in_=rep_t[128:256, g0 + HG : g0 + GROUP]
            )
            eff = io_p.tile([2, GROUP], f32r, tag="eff")
            nc.sync.dma_start(out=eff, in_=eff2[:, g0 : g0 + GROUP])
            stR = st_p.tile([128, cpg * SCOLS], f32r, tag="stR")
            nc.scalar.dma_start(
                out=stR, in_=statR[:, g * cpg * SCOLS : (g + 1) * cpg * SCOLS]
            )
            stF = st_p.tile([RZ, cpg * 2], f32, tag="stF")
            nc.scalar.dma_start(
                out=stF, in_=statF[:, g * cpg * 2 : (g + 1) * cpg * 2]
            )
            gt[g] = (rep0, rep1, eff, stR, stF)

        hrefs = {}  # chunk -> [h0, h1, h2]

        def agg_layer(c, li):
            """Stage A/B/C: one 256->256 agg layer for chunk c (4 matmuls
            into a [128,1024] 2-bank psum + one merged relu evacuation)."""
            g, j = divmod(c, cpg)
            sl = slice(j * CH, (j + 1) * CH)
            rep0, rep1, _, _, _ = gt[g]
            wname = ("w0", "w1", "w2")[li]
            ph = ph_p.tile([128, 2 * CH], f32, tag="ph")
            for mt in range(2):
                for k in range(2):
                    if li == 0:
                        rhs = (rep0, rep1)[k][:, sl]
                    else:
                        rhs = hrefs[c][li - 1][:, k * CH : (k + 1) * CH]
                    nc.tensor.matmul(
                        out=ph[:, mt * CH : (mt + 1) * CH],
                        lhsT=wk[(wname, k)][:, mt * 128 : (mt + 1) * 128],
                        rhs=rhs,
                        start=(k == 0),
                        stop=(k == 1),
                    )
            h = h_p.tile([128, 2 * CH], f32r, tag=f"h{li}")
            if split_bias:
                # general agg-bias path: per-half evacuation
                nc.scalar.activation(
                    h[:, 0:CH],
                    ph[:, 0:CH],
                    AF.Relu,
                    bias=bias_t[:, 2 * li : 2 * li + 1],
                )
                nc.vector.tensor_scalar(
                    h[:, CH : 2 * CH],
                    ph[:, CH : 2 * CH],
                    bias_t[:, 2 * li + 1 : 2 * li + 2],
                    0.0,
                    OP.add,
                    OP.max,
                )
            else:
                # zero agg-bias fast path: one [128,1024] relu
                if li == 1:
                    nc.vector.tensor_scalar(h, ph, 0.0, None, OP.max)
                else:
                    nc.scalar.activation(h, ph, AF.Relu)
            hrefs.setdefault(c, []).append(h)

        def tail(c):
            """Stage D: fused agg-layer-4 + cal-layer-1 for chunk c."""
            g, j = divmod(c, cpg)
            g0 = g * GROUP
            sl = slice(j * CH, (j + 1) * CH)
            _, _, eff, stR, stF = gt[g]
            c0 = j * SCOLS
            h3 = hrefs[c][2]
            pz = pz_p.tile([RZ, CH], f32, tag="pz")
            nc.tensor.matmul(
                out=pz,
                lhsT=stR[:, c0 : c0 + RZ],
                rhs=h3[:, 0:CH],
                start=True,
                stop=False,
            )
            nc.tensor.matmul(
                out=pz,
                lhsT=stR[:, c0 + RZ : c0 + 2 * RZ],
                rhs=h3[:, CH : 2 * CH],
                start=False,
                stop=False,
            )
            nc.tensor.matmul(
                out=pz,
                lhsT=stR[0:2, c0 + 2 * RZ : c0 + 3 * RZ],
                rhs=eff[:, sl],
                start=False,
                stop=True,
            )
            # monotone activation: per-partition clip with bias-shifted
            # bounds; logit row 24 rides through via (-BIG, BIG)
            a1 = a_p.tile([RZ, CH], f32, tag="a1")
            nc.vector.tensor_scalar(
                a1,
                pz,
                stF[:, 2 * j : 2 * j + 1],
                stF[:, 2 * j + 1 : 2 * j + 2],
                OP.max,
                OP.min,
            )
            nc.sync.dma_start(
                out=a1out[:, g0 + j * CH : g0 + (j + 1) * CH], in_=a1
            )
            del hrefs[c]

        # Depth-3 software pipeline: period j runs l0(j), tail(j-2),
        # l2(j-1), l1(j) so every relu/clip latency is hidden by
        # independent matmuls while PSUM stays within 8 banks.
        load_group(0)
        if ngroup > 1:
            load_group(1)
        for j in range(nchunk + 2):
            if j < nchunk:
                # prefetch 2 groups ahead, mid-group so the recycled slot's
                # last reader (tail of 2+ periods ago) is already emitted
                if j % cpg == 2 and (j // cpg) + 2 < ngroup:
                    load_group(j // cpg + 2)
                agg_layer(j, 0)
            if j >= 2:
                tail(j - 2)
            if 1 <= j < nchunk + 1:
                agg_layer(j - 1, 2)
            if j < nchunk:
                agg_layer(j, 1)

    nc.compile()
    return nc


def _type_templates(inputs):
    """Per-variant-type stage-1 stationaries + bias-shifted clip bounds."""
    f = np.float32
    g = lambda k: np.asarray(inputs[k], f)
    agg_W3, agg_b3 = g("agg_W3"), g("agg_b3")
    cal_W0, cal_b0 = g("cal_W0"), g("cal_b0")
    sgn_e = np.array([1.0, -1.0], f)
    opat = np.arange(12)
    lo_pat = np.where(opat < 4, 0.0, np.where(opat < 8, -BIG, -1.0)).astype(f)
    hi_pat = np.where(opat < 4, BIG, np.where(opat < 8, 0.0, 1.0)).astype(f)
    lo_z = np.concatenate([lo_pat, lo_pat, [-BIG]]).astype(f)
    hi_z = np.concatenate([hi_pat, hi_pat, [BIG]]).astype(f)

    stRs, stFs, b1s = [], [], []
    for t in range(T):
        a0 = np.abs(cal_W0[t])  # [2,12,3]
        A2 = np.zeros((F, RZ), f)
        Reff = np.zeros((2, RZ), f)
        b1 = np.zeros(RZ, f)
        for e in range(2):
            rs = slice(e * 12, e * 12 + 12)
            A2[:, rs] = agg_W3[0][:, None] * a0[e, :, 0][None, :]
            Reff[0, rs] = a0[e, :, 1] * sgn_e[e]
            Reff[1, rs] = a0[e, :, 2] * sgn_e[e]
            b1[rs] = cal_b0[t, e, :] + a0[e, :, 0] * agg_b3[0]
        A2[:, 24] = agg_W3[0]
        b1[24] = agg_b3[0]
        stR = np.zeros((128, SCOLS), f)
        stR[:, 0:RZ] = A2[0:128]
        stR[:, RZ : 2 * RZ] = A2[128:256]
        stR[0:2, 2 * RZ : 3 * RZ] = Reff
        stF = np.stack(
            [np.clip(lo_z - b1, -BIG, BIG), np.clip(hi_z - b1, -BIG, BIG)], axis=1
        ).astype(f)  # [RZ, 2]
        stRs.append(_tf32_round(stR))
        stFs.append(stF)
        b1s.append(b1)
    return stRs, stFs, np.stack(b1s)


def _host_prep(inputs):
    """Sort by type, build per-core input maps + repair metadata."""
    f = np.float32
    rep = np.asarray(inputs["representations"], f)
    refc = np.asarray(inputs["ref_counts"], f)
    altc = np.asarray(inputs["alt_counts"], f)
    vt = np.asarray(inputs["variant_types"])
    max_ref = np.asarray(inputs["max_ref"], f)
    max_alt = np.asarray(inputs["max_alt"], f)

    perm = np.argsort(vt, kind="stable")
    reps, refs, alts, vts = rep[perm], refc[perm], altc[perm], vt[perm]
    nchunk = B // CH
    tch = vts[::CH].copy()
    mixed_idx = np.where(vts != np.repeat(tch, CH))[0]

    stRs, stFs, b1s = _type_templates(inputs)
    statR = np.concatenate([stRs[t] for t in tch], axis=1)  # [128, nchunk*SCOLS]
    statF = np.concatenate([stFs[t] for t in tch], axis=1)  # [RZ, nchunk*2]

    eff2 = np.empty((2, B), f)
    eff2[0] = np.tanh(refs * np.repeat(1.0 / max_ref[tch], CH))
    eff2[1] = np.tanh(alts * np.repeat(1.0 / max_alt[tch], CH))
    eff2 = _tf32_round(eff2)

    rep_t_all = _tf32_round(np.ascontiguousarray(reps.T))

    wp = []
    for i in range(3):
        wt = _tf32_round(np.ascontiguousarray(np.asarray(inputs[f"agg_W{i}"], f).T))
        wp.append(wt[0:128])
        wp.append(wt[128:256])
    wpack = np.ascontiguousarray(np.concatenate(wp, axis=1))  # [128, 6*F]

    zero_bias = not any(np.any(np.asarray(inputs[f"agg_b{i}"])) for i in range(3))

    in_maps = []
    cpc = BS // CH  # chunks per core
    for c in range(NCORES):
        s = slice(c * BS, (c + 1) * BS)
        m = {
            "rep_t": np.ascontiguousarray(rep_t_all[:, s]),
            "eff2": np.ascontiguousarray(eff2[:, s]),
            "statR": np.ascontiguousarray(
                statR[:, c * cpc * SCOLS : (c + 1) * cpc * SCOLS]
            ),
            "statF": np.ascontiguousarray(statF[:, c * cpc * 2 : (c + 1) * cpc * 2]),
            "wpack": wpack,
        }
        if not zero_bias:
            biasw = np.zeros((128, 6), f)
            for li in range(3):
                bb = np.asarray(inputs[f"agg_b{li}"], f)
                biasw[:, 2 * li] = bb[0:128]
                biasw[:, 2 * li + 1] = bb[128:256]
            m["biasw"] = biasw
        in_maps.append(m)

    meta = dict(
        perm=perm, reps=reps, refs=refs, alts=alts, vts=vts, tch=tch, b1s=b1s,
        mixed_idx=mixed_idx, zero_bias=zero_bias,
        max_ref=max_ref, max_alt=max_alt,
    )
    return in_maps, meta


def _host_logits(inputs, rep_rows):
    f = np.float32
    h = rep_rows
    for i in range(3):
        h = np.maximum(
            h @ np.asarray(inputs[f"agg_W{i}"], f).T
            + np.asarray(inputs[f"agg_b{i}"], f),
            0.0,
        )
    return (
        h @ np.asarray(inputs["agg_W3"], f).T[:, 0]
        + np.asarray(inputs["agg_b3"], f)[0]
    )


def _host_forward_rows(inputs, meta, rows):
    """Exact fp32 reference forward for a subset of sorted-row indices."""
    f = np.float32
    lg = _host_logits(inputs, meta["reps"][rows])
    t_m = meta["vts"][rows]
    xr = np.tanh(meta["refs"][rows] / meta["max_ref"][t_m])
    xa = np.tanh(meta["alts"][rows] / meta["max_alt"][t_m])
    x = np.stack([lg, xr, xa], -1)
    signs = np.array([[1, 1, 1], [1, -1, -1]], f)
    z = x[:, None, :] * signs[None, :, :]  # [n,2,3]
    for i in range(3):
        Wc = np.abs(np.asarray(inputs[f"cal_W{i}"], f))[t_m]  # [n,2,o,i]
        bc = np.asarray(inputs[f"cal_b{i}"], f)[t_m]
        z = np.einsum("nei,neoi->neo", z, Wc) + bc
        if i < 2:
            s = z.shape[-1] // 3
            z = np.concatenate(
                [
                    np.maximum(z[..., :s], 0),
                    np.minimum(z[..., s : 2 * s], 0),
                    np.clip(z[..., 2 * s :], -1, 1),
                ],
                -1,
            )
    return np.where(lg > 0, z[:, 0, 0], z[:, 1, 0])


def _postprocess(inputs, meta, results):
    """Host cal layers 2+3 in fp32, branch-select, repairs, unsort."""
    f = np.float32
    nchunk = B // CH
    tch = meta["tch"]
    a1 = np.concatenate([r["a1"] for r in results], axis=1)  # [RZ, B]
    a1c = np.ascontiguousarray(
        a1.reshape(RZ, nchunk, CH).transpose(1, 0, 2)
    )  # [nchunk, RZ, CH]
    a1c += meta["b1s"][tch][:, :, None]  # undo bias shift

    lg = np.ascontiguousarray(a1c[:, 24, :]).reshape(-1)

    cal_W1 = np.abs(np.asarray(inputs["cal_W1"], f))  # [T,2,12,12]
    cal_b1 = np.asarray(inputs["cal_b1"], f)
    cal_W2 = np.abs(np.asarray(inputs["cal_W2"], f))  # [T,2,1,12]
    cal_b2 = np.asarray(inputs["cal_b2"], f)

    # cal layer 2: block-diagonal [24 <- 24] per type
    C2h = np.zeros((T, 24, 24), f)
    for t in range(T):
        for e in range(2):
            rs = slice(e * 12, e * 12 + 12)
            C2h[t, rs, rs] = cal_W1[t, e]  # [o,i] applied to a1 block
    z2 = np.matmul(C2h[tch], a1c[:, 0:24, :])  # [nchunk, 24, CH]
    z2 += cal_b1[tch].reshape(nchunk, 24, 1)
    # monotone activation per 12-block: 0-3 relu, 4-7 -relu(-x), 8-11 clamp
    for e in range(2):
        o = e * 12
        np.maximum(z2[:, o : o + 4], 0.0, out=z2[:, o : o + 4])
        np.minimum(z2[:, o + 4 : o + 8], 0.0, out=z2[:, o + 4 : o + 8])
        np.clip(z2[:, o + 8 : o + 12], -1.0, 1.0, out=z2[:, o + 8 : o + 12])
    # cal layer 3: [2 <- 12] per branch
    z3 = np.einsum(
        "cev,cevb->ceb", cal_W2[tch][:, :, 0, :], z2.reshape(nchunk, 2, 12, CH)
    )
    z3 += cal_b2[tch].reshape(nchunk, 2, 1)
    v0 = np.ascontiguousarray(z3[:, 0, :]).reshape(-1)
    v1 = np.ascontiguousarray(z3[:, 1, :]).reshape(-1)

    out = np.where(lg > 0.0, v0, v1).astype(f)

    # exact sign for near-zero tf32 logits
    amb = np.where(np.abs(lg) < TAU)[0]
    if amb.size:
        lgx = _host_logits(inputs, meta["reps"][amb])
        out[amb] = np.where(lgx > 0.0, v0[amb], v1[amb])

    # exact values for rows whose chunk used the wrong type's calibration
    midx = meta["mixed_idx"]
    if midx.size:
        out[midx] = _host_forward_rows(inputs, meta, midx)

    res = np.empty(B, f)
    res[meta["perm"]] = out
    return res


def _run(inputs, trace=False, tmpdir=None):
    from concourse.bass_utils import run_bass_kernel_spmd

    in_maps, meta = _host_prep(inputs)
    key = ("nc1", meta["zero_bias"])
    if key not in _CACHE:
        _CACHE[key] = build_neff1(BS, split_bias=not meta["zero_bias"])
    nc1 = _CACHE[key]
    kwargs = {}
    if tmpdir is not None:
        kwargs["tmpdir"] = tmpdir
    res1 = run_bass_kernel_spmd(
        nc1, in_maps, core_ids=list(range(NCORES)), trace=trace, **kwargs
    )
    out = _postprocess(inputs, meta, res1.results)
    return out, res1


def kernel(**inputs):
    out, _ = _run(inputs, trace=False)
    return out


if __name__ == "__main__":
    nc = build_neff1(GROUP)
    print("neff1 build ok")
